# revision 4
# baseline (speedup 1.0000x reference)
"""Trainium2 Bass kernel for nn_Attention_54142357733562 (linear/sparse attention).

Reference math (per batch b, with x flattened to [C, N]):
    Q = wq @ x ; K = wk @ x ; V = wv @ x            (1x1 convs, + zero biases)
    Qn = Q / ||Q||_c ; Kn = K / ||K||_c             (L2 norm over channel dim)
    k_sum = sum_n Kn + EPS                          [Cqk]
    tailor = 1 / (N + Qn^T k_sum)                   [N]
    kv = Kn V^T                                     [Cqk, C]
    out = (value_sum + kv^T Qn) * tailor            [C, N]

Algebraic reformulation used here (avoids materializing Qn / tailor):
    s[n]   = ||Q[:, n]||
    den[n] = N*s[n] + Q[:, n]. k_sum
    out[c,n] = (U[c,n] + value_sum[c]*s[n]) / den[n],   U = kv^T Q
which is computed as a single matmul with the scale folded into the rhs:
    Q''[m,n] = [Q; s][m,n] / den[n]      (per-n scale, applied in [n,m] layout)
    out[c,n] = sum_m [kv; value_sum][m,c] * Q''[m,n]

I/O is bf16 (host casts x down and the output back up): halves HBM traffic
vs fp32 and enables fast-weight-load on the x-stationary QKV matmuls.
All matmuls run bf16 with fp32 PSUM accumulation; the norm/denominator
chain stays fp32 in SBUF.

Sharding: 8 cores = 4 batches x 2 N-halves. Phase 1 computes per-shard
partial (kv | k_sum | value_sum) = [Kn|1]^T [V|1]; an AllReduce over the
2-core pair completes the N reduction; phase 2 computes outputs for the
shard's N range.
"""

import numpy as np
from contextlib import ExitStack

import ml_dtypes

import concourse.bass as bass
import concourse.mybir as mybir
import concourse.tile as tile
from concourse import bacc
from concourse.bass_utils import run_bass_kernel_spmd
from concourse.masks import make_identity

F32 = mybir.dt.float32
BF16 = mybir.dt.bfloat16
NP_BF16 = np.dtype(ml_dtypes.bfloat16)

C = 256
CQK = 32
J = 2 * CQK + C  # 320 = stacked [Q|K|V] output channels
EPS = 1e-6
P = 128
NT = 512  # macro-tile width along N
ST = NT // P  # 4 sub-tiles per macro


def emit_attention(tc, xs, wt, out, nsh, n_total, groups,
                   use_collective=True, phases=(1, 2)):
    """Emit the per-core SPMD program.

    xs : DRAM [C, nsh]  bf16 per-core shard of x (C-major)
    wt : DRAM [C, J]    bf16 stacked transposed weights [wq.T | wk.T | wv.T]
    out: DRAM [C, nsh]  bf16 per-core shard of the output
    """
    nc = tc.nc
    NM = nsh // NT
    SROW = nsh // P

    xs_r = xs.rearrange("(o p) n -> p o n", p=P)  # [128, 2, nsh]
    out_r = out.rearrange("(o p) n -> p o n", p=P)
    wt_r = wt.rearrange("(o p) j -> p o j", p=P)  # [128, 2, 320]

    mult = mybir.AluOpType.mult

    with ExitStack() as ctx:
        singles = ctx.enter_context(tc.tile_pool(name="singles", bufs=1))
        dram = ctx.enter_context(tc.tile_pool(name="dram", bufs=1, space="DRAM"))

        wsb = singles.tile([P, 2, J], BF16)
        nc.sync.dma_start(wsb, wt_r)
        ident = singles.tile([P, P], F32)
        make_identity(nc, ident)
        ident_b = singles.tile([P, P], BF16)
        nc.vector.tensor_copy(ident_b, ident)
        ones_b = singles.tile([P, 1], BF16)
        ones_f = singles.tile([P, 1], F32)
        nc.vector.memset(ones_f, 1.0)
        nc.vector.tensor_copy(ones_b, ones_f)

        # stash row layout (W=66): [Q 0:32 | s 32 | ||K|| 33 | K 34:66]
        # ([Q|s] contiguous at 0:33 is what phase 2 consumes)
        SW = 2 * CQK + 2
        stash = singles.tile([P, SROW, SW], F32)

        # ---------------- phase 1: QKV + partial [Kn|1]^T [V|1] ----------------
        # kvt row layout (KW=291): [V 0:256 | 1 1 | Kn 258:290 | 1]
        # matmul: lhsT = kvt[:, s, 258:291] (33 cols), rhs = kvt[:, s, 0:258]
        KW = (C + 2) + (CQK + 1)  # 258 + 33 = 291
        with ExitStack() as p1:
            xp = p1.enter_context(tc.tile_pool(name="xp", bufs=4))
            kvb = p1.enter_context(tc.tile_pool(name="kvb", bufs=6))
            scr = p1.enter_context(tc.tile_pool(name="scr", bufs=6))
            ps_qkv = p1.enter_context(tc.tile_pool(name="ps_qkv", bufs=3, space="PSUM"))
            ps_kv = p1.enter_context(tc.tile_pool(name="ps_kv", bufs=1, space="PSUM"))

            kv_acc = ps_kv.tile([P, 1, 512], F32)  # single accumulator bank

            HS = 2  # sub-tiles per half-macro (psum tile = 2 banks, bufs=3)
            for m in range(NM):
                xt = xp.tile([P, 2, NT], BF16)
                nc.sync.dma_start(xt, xs_r[:, :, m * NT:(m + 1) * NT])

                kvt = kvb.tile([P, ST, KW], BF16)
                # ones columns once per macro (GPSIMD, SBUF-only)
                nc.gpsimd.tensor_copy(
                    kvt[:, :, C:C + 2],
                    ones_b[:, None, :].to_broadcast((P, ST, 2)))
                nc.gpsimd.tensor_copy(
                    kvt[:, :, KW - 1:KW],
                    ones_b[:, None, :].to_broadcast((P, ST, 1)))

                mst = stash[:, m * ST:(m + 1) * ST, :]  # [128, 4, 66]
                for h in range(ST // HS):
                    ps = ps_qkv.tile([P, HS, 512], F32)  # 2 banks
                    for s2 in range(HS):
                        s = h * HS + s2
                        for o in range(2):
                            nc.tensor.matmul(
                                ps[:, s2, 0:J],
                                xt[:, o, s * P:(s + 1) * P],
                                wsb[:, o, :],
                                start=(o == 0),
                                stop=(o == 1),
                            )

                    r0 = m * ST + h * HS
                    st_sl = stash[:, r0:r0 + HS, :]  # [128, 2, 66]
                    kv_sl = kvt[:, h * HS:h * HS + HS, :]

                    # PSUM -> SBUF: Q,K into stash (one strided copy); V into
                    # kvt (alternating engine)
                    qk_dst = bass.AP(
                        tensor=st_sl.tensor,
                        offset=st_sl.offset,
                        ap=[st_sl.ap[0], st_sl.ap[1],
                            [CQK + 2, 2], [1, CQK]],
                    )
                    nc.vector.tensor_copy(
                        qk_dst,
                        ps[:, :, 0:2 * CQK].rearrange(
                            "p h (g c) -> p h g c", g=2))
                    if (2 * m + h) % 4 == 2:
                        nc.vector.tensor_copy(kv_sl[:, :, 0:C],
                                              ps[:, :, 2 * CQK:J])
                    else:
                        nc.scalar.copy(kv_sl[:, :, 0:C],
                                       ps[:, :, 2 * CQK:J])

                # per-macro normalization chain (batched over all 4 sub-tiles)
                qk_view = bass.AP(
                    tensor=mst.tensor,
                    offset=mst.offset,
                    ap=[mst.ap[0], mst.ap[1], [CQK + 2, 2], [1, CQK]],
                )
                sq = scr.tile([P, ST, 2, CQK], F32, tag="sq")
                nc.gpsimd.tensor_tensor(sq, qk_view, qk_view, mult)
                ssq = scr.tile([P, ST, 2], F32, tag="ssq")
                nc.vector.reduce_sum(ssq, sq, axis=mybir.AxisListType.X)
                # sqrt -> stash cols 32 (s) and 33 (||K||)
                nc.scalar.sqrt(mst[:, :, CQK:CQK + 2], ssq)
                rkn = scr.tile([P, ST, 1], F32, tag="rkn")
                nc.vector.reciprocal(rkn, mst[:, :, CQK + 1:CQK + 2])
                # Kn = K * rkn (GPSIMD, SBUF only, fp32 -> bf16)
                nc.gpsimd.tensor_tensor(
                    kvt[:, :, C + 2:C + 2 + CQK],
                    mst[:, :, CQK + 2:SW],
                    rkn.to_broadcast((P, ST, CQK)),
                    mult)

                # accumulate [Kn|1]^T [V|1|1] -> [33, 258]
                for s in range(ST):
                    nc.tensor.matmul(
                        kv_acc[0:CQK + 1, 0, 0:C + 2],
                        kvt[:, s, C + 2:KW],
                        kvt[:, s, 0:C + 2],
                        start=(m == 0 and s == 0),
                        stop=(m == NM - 1 and s == ST - 1),
                    )

            kv_sb = singles.tile([CQK + 1, C + 2], F32)
            nc.vector.tensor_copy(kv_sb, kv_acc[0:CQK + 1, 0, 0:C + 2])

        cc_in = dram.tile([CQK + 1, C + 2], F32)
        cc_out = dram.tile([CQK + 1, C + 2], F32)
        nc.sync.dma_start(cc_in, kv_sb)
        if use_collective:
            nc.gpsimd.collective_compute(
                "AllReduce",
                mybir.AluOpType.add,
                replica_groups=groups,
                ins=[cc_in.opt()],
                outs=[cc_out.opt()],
            )
        else:
            nc.sync.dma_start(cc_out, cc_in)

        # kvp[m, c]: rows 0:32 = kv, row 32 = value_sum
        kvp_f32 = singles.tile([CQK + 1, C], F32)
        nc.sync.dma_start(kvp_f32, cc_out[:, 0:C])
        kvp = singles.tile([CQK + 1, C], BF16)
        nc.vector.tensor_copy(kvp, kvp_f32)
        # ksum[p, 0:32] = k_sum + EPS (broadcast over partitions), col 32 = N
        ksum = singles.tile([P, CQK + 1], F32)
        nc.sync.dma_start(ksum[:, 0:CQK],
                          cc_out[0:CQK, C:C + 1].partition_broadcast(P))
        nc.vector.tensor_scalar_add(ksum[:, 0:CQK], ksum[:, 0:CQK], EPS)
        nc.vector.memset(ksum[:, CQK:CQK + 1], float(n_total))

        if 2 not in phases:
            # debug/measurement mode: write something so 'out' has a writer
            nc.sync.dma_start(out_r[:, :, 0:NT],
                              xs_r[:, :, 0:NT])
            return
        # ---------------- phase 2: out = [kv|vs]^T ([Q;s]/den) ----------------
        with ExitStack() as p2:
            scr2 = ctx.enter_context(tc.tile_pool(name="scr2", bufs=3))
            qtp = ctx.enter_context(tc.tile_pool(name="qtp", bufs=3))
            outp = ctx.enter_context(tc.tile_pool(name="outp", bufs=4))
            ps_qt = p2.enter_context(tc.tile_pool(name="ps_qt", bufs=2, space="PSUM"))
            ps_out = p2.enter_context(tc.tile_pool(name="ps_out", bufs=6, space="PSUM"))

            MP = 2 * ST  # process macro PAIRS: [128, 8, 33] batches
            for mp in range(NM // 2):
                st_sl = stash[:, mp * MP:(mp + 1) * MP, 0:CQK + 1]  # [128,8,33]

                prod = scr2.tile([P, MP, CQK + 1], F32, tag="prod")
                nc.gpsimd.tensor_tensor(
                    prod, st_sl,
                    ksum[:, None, :].to_broadcast((P, MP, CQK + 1)), mult)
                den = scr2.tile([P, MP, 1], F32, tag="den")
                nc.vector.reduce_sum(den, prod, axis=mybir.AxisListType.X)
                d = scr2.tile([P, MP, 1], F32, tag="d")
                nc.vector.reciprocal(d, den)
                # qsc = [Q; s] * d[n] (bf16 for 1cyc transpose + matmul)
                qsc = scr2.tile([P, MP, CQK + 1], BF16, tag="qsc")
                nc.gpsimd.tensor_tensor(qsc, st_sl,
                                        d.to_broadcast((P, MP, CQK + 1)), mult)

                qt_ps = ps_qt.tile([CQK + 1, MP, P], BF16)  # [33, 8, 128] 1 bank
                for s in range(MP):
                    nc.tensor.transpose(qt_ps[:, s, :], qsc[:, s, :], ident_b)
                qt_sb = qtp.tile([CQK + 1, MP * P], BF16)
                # per-half drains on different engines: each macro-half's
                # final matmuls wait only on their own half of qt
                nc.vector.tensor_copy(qt_sb[:, 0:MP * P // 2],
                                      qt_ps[:, 0:MP // 2, :])
                nc.scalar.copy(qt_sb[:, MP * P // 2:MP * P],
                               qt_ps[:, MP // 2:MP, :])

                for mh in range(2):
                    m = mp * 2 + mh
                    ot = outp.tile([P, 2, NT], BF16)
                    # fully independent per-block matmul->copy->DMA chains,
                    # each on its own 1-bank psum tile (6-way rotation)
                    for blk in range(2):
                        o_ps = ps_out.tile([P, NT], F32, tag="o_ps")
                        nc.tensor.matmul(
                            o_ps,
                            kvp[:, blk * P:(blk + 1) * P],
                            qt_sb[:, mh * NT:(mh + 1) * NT],
                            start=True,
                            stop=True,
                        )
                        if blk == 0:
                            nc.vector.tensor_copy(ot[:, blk, :], o_ps)
                        else:
                            nc.scalar.copy(ot[:, blk, :], o_ps)
                    nc.sync.dma_start(out_r[:, :, m * NT:(m + 1) * NT], ot)


def build_attention_nc(nsh, n_total, num_cores, groups,
                       repeat=1, use_collective=True, phases=(1, 2)):
    nc = bacc.Bacc("TRN2", target_bir_lowering=False, debug=False,
                   num_devices=num_cores)
    xs = nc.dram_tensor("xs", [C, nsh], BF16, kind="ExternalInput").ap()
    wt = nc.dram_tensor("wt", [C, J], BF16, kind="ExternalInput").ap()
    out = nc.dram_tensor("out", [C, nsh], BF16, kind="ExternalOutput").ap()
    with tile.TileContext(nc) as tc:
        for _ in range(repeat):
            emit_attention(tc, xs, wt, out, nsh, n_total, groups,
                           use_collective=use_collective, phases=phases)
    nc.compile()
    return nc


_NC_CACHE = {}


def _get_nc(nsh, n_total, num_cores, groups_key):
    key = (nsh, n_total, num_cores, groups_key)
    if key not in _NC_CACHE:
        groups = [list(g) for g in groups_key]
        _NC_CACHE[key] = build_attention_nc(nsh, n_total, num_cores, groups)
    return _NC_CACHE[key]


def _kernel_numpy(x, wq, bq, wk, bk, wv, bv):
    """Plain numpy fallback (used only for nonzero biases)."""
    b, c, h, w = x.shape
    n = h * w
    xf = x.reshape(b, c, n).astype(np.float64)
    Q = np.einsum("oc,bcn->bon", wq.astype(np.float64), xf) + bq.astype(np.float64)[None, :, None]
    K = np.einsum("oc,bcn->bon", wk.astype(np.float64), xf) + bk.astype(np.float64)[None, :, None]
    V = np.einsum("oc,bcn->bon", wv.astype(np.float64), xf) + bv.astype(np.float64)[None, :, None]
    Qn = Q / np.linalg.norm(Q, axis=1, keepdims=True)
    Kn = K / np.linalg.norm(K, axis=1, keepdims=True)
    k_sum = Kn.sum(-1) + EPS
    tailor = 1.0 / (n + np.einsum("bmn,bm->bn", Qn, k_sum))
    value_sum = V.sum(-1)
    kv = np.einsum("bmn,bcn->bmc", Kn, V)
    ms = value_sum[:, :, None] + np.einsum("bmn,bmc->bcn", Qn, kv)
    return (ms * tailor[:, None, :]).reshape(b, c, h, w).astype(np.float32)


def kernel(x, wq, bq, wk, bk, wv, bv):
    x = np.asarray(x, dtype=np.float32)
    B, Cc, H, W = x.shape
    if (any(np.any(np.asarray(b_) != 0) for b_ in (bq, bk, bv))
            or Cc != C or wq.shape != (CQK, C) or wv.shape != (C, C)
            or (H * W) % (2 * NT) != 0 or B != 4):
        return _kernel_numpy(x, wq, bq, wk, bk, wv, bv)
    N = H * W
    ncores = 8
    shards_per_batch = ncores // B  # 2
    nsh = N // shards_per_batch  # 32768
    groups_key = tuple(
        tuple(range(b * shards_per_batch, (b + 1) * shards_per_batch))
        for b in range(B)
    )

    wt = np.ascontiguousarray(
        np.concatenate([np.asarray(wq).T, np.asarray(wk).T, np.asarray(wv).T],
                       axis=1).astype(np.float32)).astype(NP_BF16)

    nc = _get_nc(nsh, N, ncores, groups_key)

    xr = x.reshape(B, Cc, N)
    in_maps = []
    for core in range(ncores):
        b, hh = core // shards_per_batch, core % shards_per_batch
        in_maps.append({
            "xs": np.ascontiguousarray(
                xr[b, :, hh * nsh:(hh + 1) * nsh]).astype(NP_BF16),
            "wt": wt,
        })

    res = run_bass_kernel_spmd(nc, in_maps, list(range(ncores)))

    out = np.empty((B, Cc, N), np.float32)
    for core in range(ncores):
        b, hh = core // shards_per_batch, core % shards_per_batch
        out[b, :, hh * nsh:(hh + 1) * nsh] = res.results[core]["out"].astype(np.float32)
    return out.reshape(B, Cc, H, W)
